# revision 13
# baseline (speedup 1.0000x reference)
"""Mixture-of-Softmaxes kernel for 8 Trainium2 NeuronCores.

Strategy: tensor-parallel over the vocab dimension (V=100000 -> 12500/core).
Each core computes all B rows for its vocab shard, HEAD-MAJOR within each
128-row block: head h's full 12500-col logit strip (fp8e4 DoubleRow matmuls,
K=256 in one PE pass) -> exp on ScalarE -> per-head row-sum on DVE ->
ONE tiny [128,1] AllReduce per (block, head) = 32 staggered collectives,
each fully off the critical path. The mixture accumulates in-place with
fused scalar_tensor_tensor (acc = e_h*w_h + acc) on DVE, deferred two
heads behind the AR trigger so collective latency never blocks the DVE
FIFO. Output gathered on host.

Key structure:
- ScalarE runs ONLY exp (+prologue tanh): no accum_out/ACCUM_READ on the
  pacing engine; row sums are 2 DVE tensor_reduces per head instead.
- emb streamed full-width per block ([128, 2, 12500] fp8, double-
  buffered): block i+1's DMA lands ~9us into block i.
- exp ring: 11 half-strip slots [128, 6356] bf16; per-head ARs free the
  previous block's slots progressively, so the next block never starves.
- 2 warmup collectives at the very top eat the one-time ~50us+15us CC
  stream setup while the prologue (input DMAs, tanh proj, pi softmax)
  runs in parallel.

Host-side prep: inputs transposed (contraction dim -> partitions); emb
pre-scaled by 16 and cast to fp8e4 (descaled for free via the exp's scale
argument); x/proj_mat/mix_mat in bf16; output bf16 -> f32 cast + vocab
concat on host.
"""

import numpy as np
import ml_dtypes

import concourse.bass as bass
import concourse.mybir as mybir
import concourse.tile as tile
from concourse import bacc
from concourse.bass_utils import run_bass_kernel_spmd
from concourse.bass_interp import get_hw_module

B, H, D, V = 1024, 4, 256, 100000
N_CORES = 8
V_S = V // N_CORES          # 12500 vocab entries per core
KT = D // 128               # 2 contraction k-tiles
N_BBLK = B // 128           # 8 b-blocks
H0W, H1W = 6144, 6356       # uneven halves of V_S (3x2048 | 3x2048+212)
SLOTW = H1W                 # ring slot width
E_SLOTS = 10                 # exp ring slots (half-strips)
# (q, offset-in-half, width): psum chunks; two [128,2048] buffers ping-pong
CHUNKS = [(0, 0, 2048), (0, 2048, 2048), (0, 4096, 2048),
          (1, 0, 2048), (1, 2048, 2048), (1, 4096, 2048), (1, 6144, 212)]
LAG = 2                     # mixture deferral in heads behind the AR

EMB_SCALE = 16.0            # host pre-scale of emb (undone in exp's scale)

F32 = mybir.dt.float32
BF16 = mybir.dt.bfloat16
FP8E4 = mybir.dt.float8e4

_RUN_KWARGS = {}  # test harness may set trace/tmpdir here
_CACHE = {}


def _build():
    nc = bacc.Bacc("TRN2", target_bir_lowering=False, debug=False,
                   num_devices=N_CORES)
    xT = nc.dram_tensor("xT", [D, B], BF16, kind="ExternalInput").ap()
    pmT = nc.dram_tensor("pmT", [D, H * D], BF16, kind="ExternalInput").ap()
    mmT = nc.dram_tensor("mmT", [D, H], BF16, kind="ExternalInput").ap()
    embT = nc.dram_tensor("embT", [128, KT * V_S], FP8E4,
                          kind="ExternalInput").ap()
    out = nc.dram_tensor("out", [B, V_S], BF16, kind="ExternalOutput").ap()

    with tile.TileContext(nc) as tc:
        _body(tc, xT, pmT, mmT, embT, out)
        tc._pool_ctx.close()

    nc.compile()
    nc.m = get_hw_module(nc.m)
    return nc


def _body(tc, xT, pmT, mmT, embT, out):
    nc = tc.nc
    Exp = mybir.ActivationFunctionType.Exp
    Tanh = mybir.ActivationFunctionType.Tanh
    add = mybir.AluOpType.add
    mult = mybir.AluOpType.mult

    import contextlib
    ctx = contextlib.ExitStack()
    tc._pool_ctx = ctx
    singles = ctx.enter_context(tc.tile_pool(name="singles", bufs=1))
    work = ctx.enter_context(tc.tile_pool(name="work", bufs=2))
    ering = ctx.enter_context(tc.tile_pool(name="ering", bufs=E_SLOTS))
    estream = ctx.enter_context(tc.tile_pool(name="estream", bufs=2))
    psum = ctx.enter_context(tc.tile_pool(name="psum", bufs=2, space="PSUM"))
    dram = ctx.enter_context(tc.tile_pool(name="dram", bufs=2, space="DRAM"))

    # ---- warm up the CC stream at the very top: the first two
    # collectives pay ~50us + ~15us of one-time setup ----
    zz = work.tile([128, 1], F32, tag="zz", name="zz")
    nc.gpsimd.memset(zz, 0.0)
    for wi in range(2):
        warm_in = dram.tile([128, 1], F32, tag=f"wrmin{wi}",
                            name=f"wrmin{wi}", bufs=1)
        warm_out = dram.tile([128, 1], F32, tag=f"wrmout{wi}",
                             name=f"wrmout{wi}", bufs=1)
        nc.gpsimd.dma_start(out=warm_in[:], in_=zz)
        nc.gpsimd.collective_compute(
            "AllReduce", add,
            replica_groups=[list(range(N_CORES))],
            ins=[warm_in.opt()], outs=[warm_out.opt()],
        )

    # ---- prologue: resident inputs ----
    sb_xT, sb_pmT, sb_mmT = [], [], []
    for k in range(KT):
        t = singles.tile([128, B], BF16, tag=f"xT{k}", name=f"xT{k}")
        nc.sync.dma_start(out=t, in_=xT[k * 128:(k + 1) * 128, :])
        sb_xT.append(t)
        t = singles.tile([128, H * D], BF16, tag=f"pmT{k}", name=f"pmT{k}")
        nc.sync.dma_start(out=t, in_=pmT[k * 128:(k + 1) * 128, :])
        sb_pmT.append(t)
        t = singles.tile([128, H], BF16, tag=f"mmT{k}", name=f"mmT{k}")
        nc.sync.dma_start(out=t, in_=mmT[k * 128:(k + 1) * 128, :])
        sb_mmT.append(t)

    ps_parity = [0]

    def next_ps():
        pstag = "psA" if ps_parity[0] % 2 == 0 else "psB"
        ps_parity[0] += 1
        return psum.tile([128, 2048], F32, tag=pstag, name=pstag, bufs=1)

    # ---- projT = tanh(proj_mat @ x.T), resident (fp8 interleaved) ----
    proj = [singles.tile([128, KT, B], FP8E4, tag=f"pj{h}", name=f"pj{h}")
            for h in range(H)]
    for h in range(H):
        for kd in range(KT):
            for bs in range(B // 512):
                ps = next_ps()
                for kc in range(KT):
                    nc.tensor.matmul(
                        ps[:, :512],
                        sb_pmT[kc][:, h * D + kd * 128:
                                    h * D + (kd + 1) * 128],
                        sb_xT[kc][:, bs * 512:(bs + 1) * 512],
                        start=(kc == 0), stop=(kc == KT - 1),
                    )
                nc.scalar.activation(
                    out=proj[h][:, kd, bs * 512:(bs + 1) * 512],
                    in_=ps[:, :512], func=Tanh)

    # ---- pi[b, h] = softmax_h(x @ mix_mat.T) per b-block ----
    # mix logits are small (|l| < ~6): exp directly, no max subtraction
    sb_pi = [None] * N_BBLK
    for i in range(N_BBLK):
        ps = next_ps()
        for kc in range(KT):
            nc.tensor.matmul(
                ps[:, :H],
                sb_xT[kc][:, i * 128:(i + 1) * 128],
                sb_mmT[kc],
                start=(kc == 0), stop=(kc == KT - 1),
            )
        e = work.tile([128, H], F32, tag="pie", name="pie")
        nc.scalar.activation(out=e, in_=ps[:, :H], func=Exp)
        s = work.tile([128, 1], F32, tag="pis", name="pis")
        nc.vector.tensor_reduce(out=s, in_=e, axis=mybir.AxisListType.X,
                                op=add)
        rs = work.tile([128, 1], F32, tag="pirs", name="pirs")
        nc.vector.reciprocal(rs, s)
        pi = singles.tile([128, H], F32, tag=f"pi{i}", name=f"pi{i}")
        nc.vector.tensor_scalar_mul(pi, e, rs)
        sb_pi[i] = pi

    # ---- main loop: head-major, head-pair AllReduces ----
    # AR j covers a pair of heads ([128,2] f32): fires after the pair's
    # sums; its AR-dependent ops (w + mixture) drain one AR later (~28us
    # of compute slack vs ~9us AR latency). The last block splits its
    # second pair into two solo ARs to shorten the tail.
    exp_scale = 1.0 / EMB_SCALE
    sums_t = [work.tile([128, H], F32, tag=f"sums{j}",
                        name=f"sums{j}") for j in range(2)]
    scratch2 = work.tile([128, 2], F32, tag="sc2", name="sc2", bufs=1)

    pending = []  # [(j, [ops])]: deferred AR-dependent ops per AR

    def drain_upto(j_now):
        while pending and pending[0][0] <= j_now - 1:
            for op in pending.pop(0)[1]:
                op()

    ar_j = [0]

    def fire_ar(i, heads, s_cols, acc, eqs, do_dma):
        """Stage + fire one AR for `heads`; queue its deferred ops."""
        j = ar_j[0]
        ar_j[0] += 1
        nh = len(heads)
        cc_in = dram.tile([128, nh], F32, tag=f"cci{j}", name=f"cci{j}",
                          bufs=1)
        cc_out = dram.tile([128, nh], F32, tag=f"cco{j}", name=f"cco{j}",
                           bufs=1)
        nc.gpsimd.dma_start(out=cc_in[:], in_=s_cols)
        nc.gpsimd.collective_compute(
            "AllReduce", add,
            replica_groups=[list(range(N_CORES))],
            ins=[cc_in.opt()], outs=[cc_out.opt()],
        )
        s_g = singles.tile([128, nh], F32, tag=f"sg{j}", name=f"sg{j}")
        rS = singles.tile([128, nh], F32, tag=f"rS{j}", name=f"rS{j}")
        w = singles.tile([128, nh], F32, tag=f"w{j}", name=f"w{j}")

        def op_w(s_g=s_g, rS=rS, w=w, cc_out=cc_out, pi=sb_pi[i],
                 h0=heads[0], nh=nh):
            nc.gpsimd.dma_start(out=s_g, in_=cc_out[:])
            nc.vector.reciprocal(rS, s_g)
            nc.vector.tensor_mul(w, pi[:, h0:h0 + nh], rS)

        ops = [op_w]
        for q, qw in ((0, H0W), (1, H1W)):
            for k, h in enumerate(heads):
                def op_mul(eq=eqs[h][q], qw=qw, w=w, k=k):
                    nc.vector.tensor_scalar_mul(eq[:, :qw], eq[:, :qw],
                                                w[:, k:k + 1])
                ops.append(op_mul)
            for h in heads:
                if h == 0:
                    continue  # head 0's slots ARE the accumulator
                def op_add(a=acc[q], eq=eqs[h][q], qw=qw):
                    nc.vector.tensor_tensor(out=a[:, :qw], in0=a[:, :qw],
                                            in1=eq[:, :qw], op=add)
                ops.append(op_add)
            if do_dma:
                def op_dma(a=acc[q], qw=qw, i=i, goff=q * H0W):
                    nc.sync.dma_start(
                        out=out[i * 128:(i + 1) * 128, goff:goff + qw],
                        in_=a[:, :qw])
                ops.append(op_dma)
        pending.append((j, ops))
        drain_upto(j)

    for i in range(N_BBLK):
        emb = estream.tile([128, KT, V_S], FP8E4, tag="emb", name=f"emb{i}")
        for kd in range(KT):
            nc.sync.dma_start(out=emb[:, kd, :],
                              in_=embT[:, kd * V_S:(kd + 1) * V_S])
        acc = [None, None]  # head-0 slots double as the mixture accumulator
        eqs = []
        last = i == N_BBLK - 1
        for h in range(H):
            eq = [ering.tile([128, SLOTW], BF16, tag="e",
                             name=f"e{i}_{h}q{q}") for q in range(2)]
            eqs.append(eq)
            if h == 0:
                acc[0], acc[1] = eq[0], eq[1]
            for (q, c0, cw) in CHUNKS:
                v0 = q * H0W + c0
                ps = next_ps()
                for ns in range((cw + 511) // 512):
                    n0 = ns * 512
                    nw = min(512, cw - n0)
                    nc.tensor.matmul(
                        ps[:, n0:n0 + nw],
                        proj[h][:, :, i * 128:(i + 1) * 128],
                        emb[:, :, v0 + n0:v0 + n0 + nw],
                        start=True, stop=True,
                        perf_mode=mybir.MatmulPerfMode.DoubleRow,
                    )
                nc.scalar.activation(out=eq[q][:, c0:c0 + cw],
                                     in_=ps[:, :cw], func=Exp,
                                     scale=exp_scale)
                # fused halving+row-sum on DVE after each half completes:
                # out=(lo+hi) (junk, scratch), accum=rowsum(out)+init.
                # q1's init chains q0's accumulator -> full head sum.
                if q == 0 and c0 + cw == H0W:
                    solo = last and h >= 2  # final block: solo ARs
                    if solo:
                        s_pair = singles.tile([128, 1], F32,
                                              tag=f"sp{i}_{h}",
                                              name=f"sp{i}_{h}")
                        col = 0
                    elif h % 2 == 0:
                        s_pair = singles.tile([128, 2], F32,
                                              tag=f"sp{i}_{h}",
                                              name=f"sp{i}_{h}")
                        col = 0
                    else:
                        col = 1
                    # 2-level halving add on Pool (idle engine), short
                    # reduce on DVE: row-sum without loading ScalarE/DVE
                    w1, w2 = H0W // 2, H0W // 4
                    scr = work.tile([128, H1W // 2], BF16, tag="scr",
                                    name="scr")
                    nc.gpsimd.tensor_tensor(
                        out=scr[:, :w1], in0=eq[0][:, :w1],
                        in1=eq[0][:, w1:H0W], op=add)
                    nc.gpsimd.tensor_tensor(
                        out=scr[:, :w2], in0=scr[:, :w2],
                        in1=scr[:, w2:w1], op=add)
                    nc.vector.tensor_reduce(
                        out=sums_t[i % 2][:, h:h + 1], in_=scr[:, :w2],
                        axis=mybir.AxisListType.X, op=add)
                elif q == 1 and c0 + cw == H1W:
                    w1, w2 = H1W // 2, H1W // 4
                    scr = work.tile([128, H1W // 2], BF16, tag="scr",
                                    name="scr")
                    nc.gpsimd.tensor_tensor(
                        out=scr[:, :w1], in0=eq[1][:, :w1],
                        in1=eq[1][:, w1:H1W], op=add)
                    nc.gpsimd.tensor_tensor(
                        out=scr[:, :w2], in0=scr[:, :w2],
                        in1=scr[:, w2:w1], op=add)
                    nc.vector.tensor_reduce(
                        out=scratch2[:, i % 2:i % 2 + 1],
                        in_=scr[:, :w2],
                        axis=mybir.AxisListType.X, op=add)
                    nc.vector.tensor_tensor(
                        out=s_pair[:, col:col + 1],
                        in0=sums_t[i % 2][:, h:h + 1],
                        in1=scratch2[:, i % 2:i % 2 + 1], op=add)
            if solo:
                fire_ar(i, [h], s_pair, acc, eqs, do_dma=(h == 3))
            elif h % 2 == 1:
                fire_ar(i, [h - 1, h], s_pair, acc, eqs,
                        do_dma=(h == 3))

    # epilogue: flush the remaining ARs' ops immediately
    drain_upto(ar_j[0] + 1)


def _get_nc():
    if "nc" not in _CACHE:
        _CACHE["nc"] = _build()
    return _CACHE["nc"]


def kernel(x, proj_mat, mix_mat, emb):
    nc = _get_nc()
    bf = ml_dtypes.bfloat16
    xT = np.ascontiguousarray(x.astype(bf).T)
    pmT = np.ascontiguousarray(proj_mat.astype(bf).T)
    mmT = np.ascontiguousarray(mix_mat.astype(bf).T)
    in_maps = []
    for c in range(N_CORES):
        shard = emb[c * V_S:(c + 1) * V_S]
        # [dl, kd*V_S + v] = emb[v, kd*128+dl] * EMB_SCALE, fp8e4
        e16 = (shard.T * EMB_SCALE).astype(ml_dtypes.float8_e4m3)
        embT = np.ascontiguousarray(
            e16.reshape(KT, 128, V_S).transpose(1, 0, 2).reshape(
                128, KT * V_S))
        in_maps.append({"xT": xT, "pmT": pmT, "mmT": mmT, "embT": embT})
    res = run_bass_kernel_spmd(nc, in_maps, list(range(N_CORES)),
                               **_RUN_KWARGS)
    _CACHE["last_result"] = res
    return np.concatenate(
        [res.results[c]["out"].astype(np.float32) for c in range(N_CORES)],
        axis=1)


# revision 16
# speedup vs baseline: 1.4335x; 1.4335x over previous
"""Mixture-of-Softmaxes kernel for 8 Trainium2 NeuronCores.

Strategy: tensor-parallel over the vocab dimension (V=100000 -> 12500/core).
Each core computes all B rows for its vocab shard, HEAD-MAJOR within each
128-row block: head h's full 12500-col logit strip (fp8e4 DoubleRow matmuls,
K=256 in one PE pass) -> exp on ScalarE -> per-head row-sum on DVE ->
ONE tiny [128,1] AllReduce per (block, head) = 32 staggered collectives,
each fully off the critical path. The mixture accumulates in-place with
fused scalar_tensor_tensor (acc = e_h*w_h + acc) on DVE, deferred two
heads behind the AR trigger so collective latency never blocks the DVE
FIFO. Output gathered on host.

Key structure:
- ScalarE runs ONLY exp (+prologue tanh): no accum_out/ACCUM_READ on the
  pacing engine; row sums are 2 DVE tensor_reduces per head instead.
- emb streamed full-width per block ([128, 2, 12500] fp8, double-
  buffered): block i+1's DMA lands ~9us into block i.
- exp ring: 11 half-strip slots [128, 6356] bf16; per-head ARs free the
  previous block's slots progressively, so the next block never starves.
- 2 warmup collectives at the very top eat the one-time ~50us+15us CC
  stream setup while the prologue (input DMAs, tanh proj, pi softmax)
  runs in parallel.

Host-side prep: inputs transposed (contraction dim -> partitions); emb
pre-scaled by 16 and cast to fp8e4 (descaled for free via the exp's scale
argument); x/proj_mat/mix_mat in bf16; output bf16 -> f32 cast + vocab
concat on host.
"""

import numpy as np
import ml_dtypes

import concourse.bass as bass
import concourse.mybir as mybir
import concourse.tile as tile
from concourse import bacc
from concourse.bass_utils import run_bass_kernel_spmd
from concourse.bass_interp import get_hw_module

B, H, D, V = 1024, 4, 256, 100000
N_CORES = 8
V_S = V // N_CORES          # 12500 vocab entries per core
KT = D // 128               # 2 contraction k-tiles
N_BBLK = B // 128           # 8 b-blocks
H0W, H1W = 6144, 6356       # uneven halves of V_S (3x2048 | 3x2048+212)
SLOTW = H1W                 # ring slot width
E_SLOTS = 10                 # exp ring slots (half-strips)
# (q, offset-in-half, width): psum chunks; two [128,2048] buffers ping-pong
CHUNKS = [(0, 0, 2048), (0, 2048, 2048), (0, 4096, 2048),
          (1, 0, 2048), (1, 2048, 2048), (1, 4096, 2048), (1, 6144, 212)]
NCH = len(CHUNKS)           # chunks (= accum cols) per head

EMB_SCALE = 16.0            # host pre-scale of emb (undone in exp's scale)

F32 = mybir.dt.float32
BF16 = mybir.dt.bfloat16
FP8E4 = mybir.dt.float8e4

_RUN_KWARGS = {}  # test harness may set trace/tmpdir here
_CACHE = {}


def _build():
    nc = bacc.Bacc("TRN2", target_bir_lowering=False, debug=False,
                   num_devices=N_CORES)
    xT = nc.dram_tensor("xT", [D, B], BF16, kind="ExternalInput").ap()
    pmT = nc.dram_tensor("pmT", [D, H * D], BF16, kind="ExternalInput").ap()
    mmT = nc.dram_tensor("mmT", [D, H], BF16, kind="ExternalInput").ap()
    embT = nc.dram_tensor("embT", [128, KT * V_S], FP8E4,
                          kind="ExternalInput").ap()
    out = nc.dram_tensor("out", [B, V_S], BF16, kind="ExternalOutput").ap()

    with tile.TileContext(nc) as tc:
        _body(tc, xT, pmT, mmT, embT, out)
        tc._pool_ctx.close()

    nc.compile()
    nc.m = get_hw_module(nc.m)
    return nc


def _body(tc, xT, pmT, mmT, embT, out):
    nc = tc.nc
    Exp = mybir.ActivationFunctionType.Exp
    Tanh = mybir.ActivationFunctionType.Tanh
    add = mybir.AluOpType.add
    mult = mybir.AluOpType.mult

    import contextlib
    ctx = contextlib.ExitStack()
    tc._pool_ctx = ctx
    singles = ctx.enter_context(tc.tile_pool(name="singles", bufs=1))
    work = ctx.enter_context(tc.tile_pool(name="work", bufs=2))
    ering = ctx.enter_context(tc.tile_pool(name="ering", bufs=E_SLOTS))
    estream = ctx.enter_context(tc.tile_pool(name="estream", bufs=2))
    psum = ctx.enter_context(tc.tile_pool(name="psum", bufs=2, space="PSUM"))
    dram = ctx.enter_context(tc.tile_pool(name="dram", bufs=2, space="DRAM"))

    # ---- warm up the CC stream at the very top: the first two
    # collectives pay ~50us + ~15us of one-time setup ----
    zz = work.tile([128, 1], F32, tag="zz", name="zz")
    nc.gpsimd.memset(zz, 0.0)
    for wi in range(2):
        warm_in = dram.tile([128, 1], F32, tag=f"wrmin{wi}",
                            name=f"wrmin{wi}", bufs=1)
        warm_out = dram.tile([128, 1], F32, tag=f"wrmout{wi}",
                             name=f"wrmout{wi}", bufs=1)
        nc.gpsimd.dma_start(out=warm_in[:], in_=zz)
        nc.gpsimd.collective_compute(
            "AllReduce", add,
            replica_groups=[list(range(N_CORES))],
            ins=[warm_in.opt()], outs=[warm_out.opt()],
        )

    # ---- prologue: resident inputs ----
    sb_xT, sb_pmT, sb_mmT = [], [], []
    for k in range(KT):
        t = singles.tile([128, B], BF16, tag=f"xT{k}", name=f"xT{k}")
        nc.sync.dma_start(out=t, in_=xT[k * 128:(k + 1) * 128, :])
        sb_xT.append(t)
        t = singles.tile([128, H * D], BF16, tag=f"pmT{k}", name=f"pmT{k}")
        nc.sync.dma_start(out=t, in_=pmT[k * 128:(k + 1) * 128, :])
        sb_pmT.append(t)
        t = singles.tile([128, H], BF16, tag=f"mmT{k}", name=f"mmT{k}")
        nc.sync.dma_start(out=t, in_=mmT[k * 128:(k + 1) * 128, :])
        sb_mmT.append(t)

    ps_parity = [0]

    def next_ps():
        pstag = "psA" if ps_parity[0] % 2 == 0 else "psB"
        ps_parity[0] += 1
        return psum.tile([128, 2048], F32, tag=pstag, name=pstag, bufs=1)

    # ---- projT = tanh(proj_mat @ x.T), resident (fp8 interleaved) ----
    proj = [singles.tile([128, KT, B], FP8E4, tag=f"pj{h}", name=f"pj{h}")
            for h in range(H)]
    for h in range(H):
        for kd in range(KT):
            for bs in range(B // 512):
                ps = next_ps()
                for kc in range(KT):
                    nc.tensor.matmul(
                        ps[:, :512],
                        sb_pmT[kc][:, h * D + kd * 128:
                                    h * D + (kd + 1) * 128],
                        sb_xT[kc][:, bs * 512:(bs + 1) * 512],
                        start=(kc == 0), stop=(kc == KT - 1),
                    )
                nc.scalar.activation(
                    out=proj[h][:, kd, bs * 512:(bs + 1) * 512],
                    in_=ps[:, :512], func=Tanh)

    # ---- pi[b, h] = softmax_h(x @ mix_mat.T) per b-block ----
    # mix logits are small (|l| < ~6): exp directly, no max subtraction
    sb_pi = [None] * N_BBLK
    for i in range(N_BBLK):
        ps = next_ps()
        for kc in range(KT):
            nc.tensor.matmul(
                ps[:, :H],
                sb_xT[kc][:, i * 128:(i + 1) * 128],
                sb_mmT[kc],
                start=(kc == 0), stop=(kc == KT - 1),
            )
        e = work.tile([128, H], F32, tag="pie", name="pie")
        nc.scalar.activation(out=e, in_=ps[:, :H], func=Exp)
        s = work.tile([128, 1], F32, tag="pis", name="pis")
        nc.vector.tensor_reduce(out=s, in_=e, axis=mybir.AxisListType.X,
                                op=add)
        rs = work.tile([128, 1], F32, tag="pirs", name="pirs")
        nc.vector.reciprocal(rs, s)
        pi = singles.tile([128, H], F32, tag=f"pi{i}", name=f"pi{i}")
        nc.vector.tensor_scalar_mul(pi, e, rs)
        sb_pi[i] = pi

    # ---- main loop: head-major, head-pair AllReduces ----
    # AR j covers a pair of heads ([128,2] f32): fires after the pair's
    # sums; its AR-dependent ops (w + mixture) drain one AR later (~28us
    # of compute slack vs ~9us AR latency). The last block splits its
    # second pair into two solo ARs to shorten the tail.
    exp_scale = 1.0 / EMB_SCALE
    sums_t = [work.tile([128, H * NCH], F32, tag=f"sums{j}",
                        name=f"sums{j}") for j in range(2)]

    pending = []  # [(j, [ops])]: deferred AR-dependent ops per AR

    def drain_upto(j_now):
        while pending and pending[0][0] <= j_now - 1:
            for op in pending.pop(0)[1]:
                op()

    ar_j = [0]

    def fire_ar(i, heads, s_cols, acc, eqs, do_dma):
        """Stage + fire one AR for `heads`; queue its deferred ops."""
        j = ar_j[0]
        ar_j[0] += 1
        nh = len(heads)
        cc_in = dram.tile([128, nh], F32, tag=f"cci{j}", name=f"cci{j}",
                          bufs=1)
        cc_out = dram.tile([128, nh], F32, tag=f"cco{j}", name=f"cco{j}",
                           bufs=1)
        nc.gpsimd.dma_start(out=cc_in[:], in_=s_cols)
        nc.gpsimd.collective_compute(
            "AllReduce", add,
            replica_groups=[list(range(N_CORES))],
            ins=[cc_in.opt()], outs=[cc_out.opt()],
        )
        s_g = singles.tile([128, nh], F32, tag=f"sg{j}", name=f"sg{j}")
        rS = singles.tile([128, nh], F32, tag=f"rS{j}", name=f"rS{j}")
        w = singles.tile([128, nh], F32, tag=f"w{j}", name=f"w{j}")

        def op_w(s_g=s_g, rS=rS, w=w, cc_out=cc_out, pi=sb_pi[i],
                 h0=heads[0], nh=nh):
            nc.gpsimd.dma_start(out=s_g, in_=cc_out[:])
            nc.vector.reciprocal(rS, s_g)
            nc.vector.tensor_mul(w, pi[:, h0:h0 + nh], rS)

        ops = [op_w]
        for q, qw in ((0, H0W), (1, H1W)):
            for k, h in enumerate(heads):
                def op_mul(eq=eqs[h][q], qw=qw, w=w, k=k):
                    nc.vector.tensor_scalar_mul(eq[:, :qw], eq[:, :qw],
                                                w[:, k:k + 1])
                ops.append(op_mul)
            for h in heads:
                if h == 0:
                    continue  # head 0's slots ARE the accumulator
                def op_add(a=acc[q], eq=eqs[h][q], qw=qw):
                    nc.vector.tensor_tensor(out=a[:, :qw], in0=a[:, :qw],
                                            in1=eq[:, :qw], op=add)
                ops.append(op_add)
            if do_dma:
                def op_dma(a=acc[q], qw=qw, i=i, goff=q * H0W):
                    nc.sync.dma_start(
                        out=out[i * 128:(i + 1) * 128, goff:goff + qw],
                        in_=a[:, :qw])
                ops.append(op_dma)
        pending.append((j, ops))
        drain_upto(j)

    for i in range(N_BBLK):
        emb = estream.tile([128, KT, V_S], FP8E4, tag="emb", name=f"emb{i}")
        for kd in range(KT):
            nc.sync.dma_start(out=emb[:, kd, :],
                              in_=embT[:, kd * V_S:(kd + 1) * V_S])
        acc = [None, None]  # head-0 slots double as the mixture accumulator
        eqs = []
        last = i == N_BBLK - 1
        for h in range(H):
            eq = [ering.tile([128, SLOTW], BF16, tag="e",
                             name=f"e{i}_{h}q{q}") for q in range(2)]
            eqs.append(eq)
            if h == 0:
                acc[0], acc[1] = eq[0], eq[1]
            for cidx, (q, c0, cw) in enumerate(CHUNKS):
                v0 = q * H0W + c0
                ps = next_ps()
                for ns in range((cw + 511) // 512):
                    n0 = ns * 512
                    nw = min(512, cw - n0)
                    nc.tensor.matmul(
                        ps[:, n0:n0 + nw],
                        proj[h][:, :, i * 128:(i + 1) * 128],
                        emb[:, :, v0 + n0:v0 + n0 + nw],
                        start=True, stop=True,
                        perf_mode=mybir.MatmulPerfMode.DoubleRow,
                    )
                dst = eq[q][:, c0:c0 + cw]
                scol = sums_t[i % 2][:, h * NCH + cidx:h * NCH + cidx + 1]
                if q == 0:
                    # row-sum rides the ACT accumulator: zero extra
                    # SBUF traffic, 300ns ACCUM_READ on ScalarE
                    nc.scalar.activation(out=dst, in_=ps[:, :cw],
                                         func=Exp, scale=exp_scale,
                                         accum_out=scol)
                else:
                    # q1 row-sums: DVE in-place pseudo-copy accumulator
                    nc.scalar.activation(out=dst, in_=ps[:, :cw],
                                         func=Exp, scale=exp_scale)
                    nc.vector.tensor_scalar(out=dst, in0=dst,
                                            scalar1=1.0, scalar2=None,
                                            op0=mult, op1=add,
                                            accum_out=scol)
            solo = last and h >= 2  # final block: solo ARs for heads 2, 3
            if solo:
                s_pair = singles.tile([128, 1], F32, tag=f"sp{i}_{h}",
                                      name=f"sp{i}_{h}")
                col = 0
            elif h % 2 == 0:
                s_pair = singles.tile([128, 2], F32, tag=f"sp{i}_{h}",
                                      name=f"sp{i}_{h}")
                col = 0
            else:
                col = 1
            nc.vector.tensor_reduce(
                out=s_pair[:, col:col + 1],
                in_=sums_t[i % 2][:, h * NCH:(h + 1) * NCH],
                axis=mybir.AxisListType.X, op=add)
            if solo:
                fire_ar(i, [h], s_pair, acc, eqs, do_dma=(h == 3))
            elif h % 2 == 1:
                fire_ar(i, [h - 1, h], s_pair, acc, eqs,
                        do_dma=(h == 3))

    # epilogue: flush the remaining ARs' ops immediately
    drain_upto(ar_j[0] + 1)


def _get_nc():
    if "nc" not in _CACHE:
        _CACHE["nc"] = _build()
    return _CACHE["nc"]


def kernel(x, proj_mat, mix_mat, emb):
    nc = _get_nc()
    bf = ml_dtypes.bfloat16
    xT = np.ascontiguousarray(x.astype(bf).T)
    pmT = np.ascontiguousarray(proj_mat.astype(bf).T)
    mmT = np.ascontiguousarray(mix_mat.astype(bf).T)
    in_maps = []
    for c in range(N_CORES):
        shard = emb[c * V_S:(c + 1) * V_S]
        # [dl, kd*V_S + v] = emb[v, kd*128+dl] * EMB_SCALE, fp8e4
        e16 = (shard.T * EMB_SCALE).astype(ml_dtypes.float8_e4m3)
        embT = np.ascontiguousarray(
            e16.reshape(KT, 128, V_S).transpose(1, 0, 2).reshape(
                128, KT * V_S))
        in_maps.append({"xT": xT, "pmT": pmT, "mmT": mmT, "embT": embT})
    res = run_bass_kernel_spmd(nc, in_maps, list(range(N_CORES)),
                               **_RUN_KWARGS)
    _CACHE["last_result"] = res
    return np.concatenate(
        [res.results[c]["out"].astype(np.float32) for c in range(N_CORES)],
        axis=1)
